# revision 1
# baseline (speedup 1.0000x reference)
"""CostVolume2D Trainium2 kernel.

out[b, d, h, w] = mean_c l[b,c,h,w] * r_pad[b,c,h, w + maxd - (d - maxd)]
               = mean_c l[b,c,h,w] * r[b,c,h, w - (d - maxd)]   (zero padded)

Strategy (8 NeuronCores, shard H — no halo since shifts only touch W):
  * Per (b, h): the 97 disparity planes are the diagonals of banded gram
    matrices G[w, w'] = sum_c l[c, w] r[c, w'] with |w - w'| <= 48.
  * Tensor engine computes G in [128 x 256] blocks (f32r, N=256 -> full rate):
      lhsT = l[:, w0:w0+128]  (K=64 channels on partitions)
      rhs  = r_padded[:, w0-48 : w0+208]
  * DVE evicts the needed 224 columns of each PSUM block to SBUF.
  * The skew (diagonal extraction) rides the store DMA: row i of a gram
    block holds the 97 output values for w = w0 + i *contiguously*
    (cols [i, i+97)), so a DMA with a joint partition+byte stride
    (flat stride = row_pitch + 1) writes output laid out as
    O[b, h, w, k] with k = maxd*2 - d_idx. Host unshards with a
    flip + transpose (pure layout glue).
  * Host pre-divides l by C (exact, power of two) so no on-device scaling,
    and pre-pads r along W so no on-device memset / edge handling.
"""

import sys

try:
    import concourse  # noqa: F401
except ImportError:
    sys.path.insert(0, "/opt/trn_rl_repo")

import numpy as np

from concourse import bass, mybir
from concourse import tile
from concourse.ap import AP
from concourse.bass_utils import run_bass_kernel_spmd

F32 = mybir.dt.float32
F32R = mybir.dt.float32r
F16 = mybir.dt.float16

# Problem dims (hardcoded per spec)
B, C, H, W = 4, 64, 256, 512
MAXD = 48
D = 2 * MAXD + 1          # 97 disparity planes
NCORES = 8
HS = H // NCORES          # 32 h-rows per core

# Derived tiling constants
WB = 128                  # w-block (gram rows per block)
NQ = W // WB              # 4 w-blocks
GW = WB + 2 * MAXD        # 224 gram columns per block
NMM = GW                  # matmul moving dim (bf16: no f32r N>=256 rule)
RPAD_L = MAXD             # left zero pad of r
RPAD_R = NMM - WB - MAXD  # 48: right pad so q=3's window is in bounds
WP = W + RPAD_L + RPAD_R  # 640 padded r width
HGRP = 8                  # h-rows loaded per input DMA (must divide HS, %2==0)
OROW = 1024               # out slots per w-row (written with pitch OROW-1)

# module-level result stash (test.py reads these)
LAST_RESULTS = None
_NC_CACHE = {}


WLR = W + WP              # 1152: combined (l | r_pad) row width


def _build_nc(b_n=B, hs=HS, hgrp=HGRP):
    """Build the per-core Bass program. All cores run the same program."""
    assert hs % hgrp == 0 and hgrp % 2 == 0
    nc = bass.Bass()
    # l and r_pad concatenated on the W axis -> ONE load DMA per h-half,
    # so every matmul depends on a single DMA semaphore lane (the f32r
    # self-loading Matmult instruction only has room for one sync wait).
    lr_in = nc.dram_tensor("lr", [b_n, C, hs, WLR], F16, kind="ExternalInput")
    o_out = nc.dram_tensor("o", [b_n, hs, WB, OROW], F16, kind="ExternalOutput")

    lr_c, lr_h = hs * WLR, WLR
    lr_b = C * hs * WLR

    n4 = hgrp // 2            # h-pairs per group
    lrw = n4 * WLR            # free width of lr tile
    gp_pitch = NQ * GW        # 896: g tile row pitch

    with tile.TileContext(nc) as tc:
        with (
            tc.tile_pool(name="lrpool", bufs=3) as lrp,
            tc.tile_pool(name="gpool", bufs=8) as gp,
            tc.tile_pool(name="ppool", bufs=8, space="PSUM") as pp,
        ):
            for b in range(b_n):
                for hg in range(hs // hgrp):
                    h0 = hg * hgrp
                    lr_t = lrp.tile([128, lrw], F16, name="lr_t")
                    # partitions = (hh in 2) x (c in 64); free = (h4, w_lr)
                    # DMA APs are limited to 3 dims -> one DMA per hh half.
                    for hh in range(2):
                        lr_src = AP(
                            lr_in, b * lr_b + (h0 + hh) * lr_h,
                            [(lr_c, C), (2 * lr_h, n4), (1, WLR)],
                        )
                        nc.sync.dma_start(
                            out=lr_t[64 * hh:64 * hh + 64, :], in_=lr_src
                        )
                    for h4 in range(n4):
                        g0 = gp.tile([128, gp_pitch], F16, name="g0", tag="g")
                        g1 = gp.tile([128, gp_pitch], F16, name="g1", tag="g")
                        gs = (g0, g1)
                        for qp in range(NQ // 2):
                            for hh in range(2):
                                p_t = pp.tile([128, 2 * NMM], F32, name="p_t")
                                for qq in range(2):
                                    q = 2 * qp + qq
                                    lhsT = lr_t[
                                        64 * hh:64 * hh + 64,
                                        h4 * WLR + WB * q:
                                        h4 * WLR + WB * q + WB,
                                    ]
                                    rhs = lr_t[
                                        64 * hh:64 * hh + 64,
                                        h4 * WLR + W + WB * q:
                                        h4 * WLR + W + WB * q + NMM,
                                    ]
                                    nc.tensor.matmul(
                                        p_t[:, NMM * qq:NMM * qq + NMM],
                                        lhsT, rhs, start=True, stop=True,
                                    )
                                nc.vector.tensor_copy(
                                    gs[hh][:, 2 * GW * qp: 2 * GW * qp + 2 * GW],
                                    p_t[:],
                                )
                        for hh in range(2):
                            h = h0 + 2 * h4 + hh
                            g = gs[hh]
                            # Full-row skew store: one descriptor per gram row
                            # (3584B). DRAM row pitch is OROW-1 elements, so
                            # row i's data lands shifted by -i: the diagonal
                            # relayout happens in the DRAM addressing, and the
                            # writes tile the region exactly (no overlap).
                            # Valid data sits at fixed slots 127+224q+k'.
                            d_ap = AP(
                                o_out,
                                (b * hs + h) * WB * OROW + (WB - 1),
                                [(OROW - 1, WB), (1, gp_pitch)],
                            )
                            eng = nc.sync if (hh % 2) else nc.scalar
                            eng.dma_start(out=d_ap, in_=g[:, :])
    _split_multi_waits(nc)
    return nc


def _split_multi_waits(nc):
    """The 64-byte TPB instruction encoding holds a single semaphore wait;
    walrus codegen rejects instructions whose sync_info carries more. Hoist
    all but one wait onto standalone InstEventSemaphore instructions placed
    immediately before, on the same engine (FIFO order preserves semantics).
    """
    for bb in nc.main_func.blocks:
        new_list = []
        changed = False
        for ins in bb.instructions:
            si = ins.sync_info
            if si is not None and len(si.on_wait) > 1:
                for w in list(si.on_wait)[:-1]:
                    ev = mybir.InstEventSemaphore(
                        name=nc.get_next_instruction_name(),
                        engine=ins.engine,
                        ins=[],
                        outs=[],
                        sync_info=mybir.SyncInfo(on_wait=[w], on_update=[]),
                    )
                    new_list.append(ev)
                ins.sync_info = mybir.SyncInfo(
                    on_wait=[list(si.on_wait)[-1]], on_update=list(si.on_update)
                )
                changed = True
            new_list.append(ins)
        if changed:
            bb.instructions = new_list


def _get_nc(key=(B, HS, HGRP)):
    if key not in _NC_CACHE:
        _NC_CACHE[key] = _build_nc(*key)
    return _NC_CACHE[key]


def _host_prep(l_fmap, r_fmap):
    l = np.asarray(l_fmap, dtype=np.float32)
    r = np.asarray(r_fmap, dtype=np.float32)
    l = l * np.float32(1.0 / C)  # exact: C is a power of two
    lr = np.empty(l.shape[:3] + (WLR,), dtype=np.float16)
    lr[..., :W] = l
    lr[..., W + RPAD_L:W + RPAD_L + W] = r
    lr[..., W:W + RPAD_L] = 0.0
    lr[..., W + RPAD_L + W:] = 0.0
    return lr


def _install_ntff_hook_shim(so_path="/opt/axon/libaxon_pjrt.so"):
    """Provide antenv.axon_hooks.get_axon_ntff_profile_hook via ctypes when
    the image's antenv lacks it (mirrors trn_agent_boot's slim hook)."""
    import types
    import ctypes
    import contextlib

    try:
        from antenv.axon_hooks import get_axon_ntff_profile_hook  # noqa: F401
        return
    except ImportError:
        pass

    lib = ctypes.CDLL(so_path)
    if not hasattr(lib, "axon_start_nrt_profile"):
        return
    lib.axon_start_nrt_profile.argtypes = [
        ctypes.POINTER(ctypes.c_int64), ctypes.c_size_t,
    ]
    lib.axon_start_nrt_profile.restype = ctypes.c_int64
    lib.axon_stop_nrt_profile.argtypes = [ctypes.c_char_p]
    lib.axon_stop_nrt_profile.restype = ctypes.c_int64

    @contextlib.contextmanager
    def _hook(output_dir, device_ids):
        import jax
        jax.devices()
        if device_ids:
            ids = (ctypes.c_int64 * len(device_ids))(*device_ids)
            rc = lib.axon_start_nrt_profile(ids, len(device_ids))
        else:
            rc = lib.axon_start_nrt_profile(None, 0)
        if rc != 0:
            raise RuntimeError(f"axon_start_nrt_profile rc={rc}")
        try:
            yield
        finally:
            n = lib.axon_stop_nrt_profile(str(output_dir).encode())
            print(f"ntff profile: {n} file(s) written to {output_dir}",
                  file=sys.stderr)

    import antenv
    mod = types.ModuleType("antenv.axon_hooks")
    mod.get_axon_ntff_profile_hook = lambda: _hook
    mod.set_axon_ntff_profile_hook = lambda h: None
    sys.modules["antenv.axon_hooks"] = mod
    antenv.axon_hooks = mod


def kernel(l_fmap, r_fmap, max_disp):
    global LAST_RESULTS
    assert int(max_disp) == MAXD
    lr = _host_prep(l_fmap, r_fmap)
    assert lr.shape == (B, C, H, WLR)

    nc = _get_nc()
    in_maps = []
    for k in range(NCORES):
        sl = slice(k * HS, (k + 1) * HS)
        in_maps.append({
            "lr": np.ascontiguousarray(lr[:, :, sl, :]),
        })

    import os
    trace = bool(int(os.environ.get("CV_TRACE", "0")))
    if trace:
        _install_ntff_hook_shim()
    res = run_bass_kernel_spmd(nc, in_maps, list(range(NCORES)), trace=trace)
    LAST_RESULTS = res

    out = np.empty((B, D, H, W), dtype=np.float32)
    for k in range(NCORES):
        o = np.asarray(res.results[k]["o"]).astype(np.float32)  # [B,HS,WB,OROW]
        o5 = np.stack(
            [o[..., 127 + GW * q:127 + GW * q + D] for q in range(NQ)], axis=2
        )  # [B, HS, NQ, WB, D]
        # out[b, 96-k', h, 128q+i] = o5[b, h, q, i, k']
        tmp = np.flip(o5, axis=4).transpose(0, 4, 1, 2, 3)  # [B,D,HS,NQ,WB]
        out[:, :, k * HS:(k + 1) * HS, :] = tmp.reshape(B, D, HS, W)
    return out



# revision 2
# speedup vs baseline: 1.0038x; 1.0038x over previous
"""CostVolume2D Trainium2 kernel (v2).

out[b, d, h, w] = mean_c l[b,c,h,w] * r[b,c,h, w - (d - maxd)]   (zero padded)

Strategy (8 NeuronCores, shard H — no halo since shifts only touch W):
  * Per (b, h): the 97 disparity planes are the diagonals of banded gram
    blocks G_q[i, n] = sum_c l[c, 128q+i] r_pad[c, 128q-48+n], n in [0,224).
  * Tensor engine: 4 matmuls per h (K=64 channels, M=128 w's, N=224).
  * Eviction (PSUM -> SBUF, f32 -> f16) INTERLEAVES the 4 q-blocks:
    g[i, 896*t + 4n + q] = G_q[i, n] for h-in-group t. Row i's 388 valid
    values (the 97-band of all 4 q's) then occupy CONSECUTIVE columns
    [4i, 4i+388) of its 896-column block. Evictions alternate DVE /
    Activation engines to halve per-engine time.
  * Store: per 32-row sub-block m, the valid windows of rows
    [32m, 32m+32) all sit inside columns [128m, 128m+512) — a plain
    rectangular slice. One 3-dim DMA per (group, m) writes
    [32 rows x 4 h x 512] with 1024B descriptors: 16.8 MB/core instead
    of the 29.4 MB full-gram store (valid data is 12.7 MB).
  * Input: l and zero-padded r are packed per row as [l 512 | r_pad 608]
    and laid out so each partition (hh, c) reads its 16 rows contiguously:
    one load DMA per b with 35,840-byte descriptors.
  * Host pre-divides l by C (exact, power of two); host unshard is a
    strided view + transpose (pure layout glue).
"""

import sys

try:
    import concourse  # noqa: F401
except ImportError:
    sys.path.insert(0, "/opt/trn_rl_repo")

import numpy as np

from concourse import bass, mybir
from concourse import tile
from concourse.ap import AP
from concourse.bass_utils import run_bass_kernel_spmd

F32 = mybir.dt.float32
F16 = mybir.dt.float16

# Problem dims (hardcoded per spec)
B, C, H, W = 4, 64, 256, 512
MAXD = 48
D = 2 * MAXD + 1          # 97 disparity planes
NCORES = 8
HS = H // NCORES          # 32 h-rows per core

# Tiling
NH4 = HS // 2             # 16 h-pairs per core (partition dim packs hh in 2)
WROW = W + W + 2 * MAXD   # 1120: [l 512 | r_pad 608] per (c, h) row
NHG = HS // 4             # 8 groups of 4 h per g-tile
GPW = 4 * 224             # 896 interleaved gram columns per h
NM = 4                    # 32-row store sub-blocks
SBW = 512                 # stored row width per sub-block (388 valid + skew)

LAST_RESULTS = None
_NC_CACHE = {}


def _build_nc():
    nc = bass.Bass()
    lr_in = nc.dram_tensor("lr", [B, 2, C, NH4, WROW], F16, kind="ExternalInput")
    o_out = nc.dram_tensor(
        "o", [B, NHG, NM, 32, 4, SBW], F16, kind="ExternalOutput"
    )
    lrw = NH4 * WROW      # 17920 free width of lr tile

    with tile.TileContext(nc) as tc:
        with (
            tc.tile_pool(name="lrpool", bufs=2) as lrp,
            tc.tile_pool(name="gpool", bufs=4) as gp,
            tc.tile_pool(name="ppool", bufs=8, space="PSUM") as pp,
        ):
            for b in range(B):
                lr_t = lrp.tile([128, lrw], F16, name="lr_t")
                nc.scalar.dma_start(
                    out=lr_t[:, :],
                    in_=AP(lr_in, b * 2 * C * lrw, [(lrw, 128), (1, lrw)]),
                )
                for hg in range(NHG):
                    g = gp.tile([128, 4 * GPW], F16, name="g", tag="g")
                    for t in range(4):
                        h4 = 2 * hg + (t >> 1)
                        hh = t & 1
                        cb = h4 * WROW
                        for qp in range(2):
                            p_t = pp.tile([128, 448], F32, name="p_t")
                            for qq in range(2):
                                q = 2 * qp + qq
                                lhsT = lr_t[
                                    64 * hh:64 * hh + 64,
                                    cb + 128 * q:cb + 128 * q + 128,
                                ]
                                rhs = lr_t[
                                    64 * hh:64 * hh + 64,
                                    cb + W + 128 * q:cb + W + 128 * q + 224,
                                ]
                                nc.tensor.matmul(
                                    p_t[:, 224 * qq:224 * qq + 224],
                                    lhsT, rhs, start=True, stop=True,
                                )
                            src = AP(
                                p_t.tensor, 0, [(448, 128), (224, 2), (1, 224)]
                            )
                            dst = AP(
                                g.tensor, GPW * t + 2 * qp,
                                [(4 * GPW, 128), (1, 2), (4, 224)],
                            )
                            if qp == 0:
                                nc.vector.tensor_copy(dst, src)
                            else:
                                nc.scalar.copy(dst, src)
                    for m in range(NM):
                        nc.sync.dma_start(
                            out=AP(
                                o_out,
                                ((b * NHG + hg) * NM + m) * 32 * 4 * SBW,
                                [(4 * SBW, 32), (SBW, 4), (1, SBW)],
                            ),
                            in_=AP(
                                g.tensor,
                                m * (32 * 4 * GPW + 128),
                                [(4 * GPW, 32), (GPW, 4), (1, SBW)],
                            ),
                        )
    _split_multi_waits(nc)
    return nc


def _split_multi_waits(nc):
    """The 64-byte TPB instruction encoding holds a single semaphore wait;
    walrus codegen rejects instructions whose sync_info carries more. Hoist
    all but one wait onto standalone InstEventSemaphore instructions placed
    immediately before, on the same engine (FIFO order preserves semantics).
    """
    for bb in nc.main_func.blocks:
        new_list = []
        changed = False
        for ins in bb.instructions:
            si = ins.sync_info
            if si is not None and len(si.on_wait) > 1:
                for w in list(si.on_wait)[:-1]:
                    ev = mybir.InstEventSemaphore(
                        name=nc.get_next_instruction_name(),
                        engine=ins.engine,
                        ins=[],
                        outs=[],
                        sync_info=mybir.SyncInfo(on_wait=[w], on_update=[]),
                    )
                    new_list.append(ev)
                ins.sync_info = mybir.SyncInfo(
                    on_wait=[list(si.on_wait)[-1]], on_update=list(si.on_update)
                )
                changed = True
            new_list.append(ins)
        if changed:
            bb.instructions = new_list


def _get_nc():
    if "nc" not in _NC_CACHE:
        _NC_CACHE["nc"] = _build_nc()
    return _NC_CACHE["nc"]


def _host_prep(l_fmap, r_fmap):
    l = np.asarray(l_fmap, dtype=np.float32) * np.float32(1.0 / C)
    r = np.asarray(r_fmap, dtype=np.float32)
    # per-core layout [k, b, hh, c, h4, col]; h_global = 32k + 2*h4 + hh
    lr = np.zeros((NCORES, B, 2, C, NH4, WROW), dtype=np.float16)
    l6 = l.reshape(B, C, NCORES, NH4, 2, W).transpose(2, 0, 4, 1, 3, 5)
    r6 = r.reshape(B, C, NCORES, NH4, 2, W).transpose(2, 0, 4, 1, 3, 5)
    lr[..., 0:W] = l6
    lr[..., W + MAXD:W + MAXD + W] = r6
    return lr


def _install_ntff_hook_shim(so_path="/opt/axon/libaxon_pjrt.so"):
    """Provide antenv.axon_hooks.get_axon_ntff_profile_hook via ctypes when
    the image's antenv lacks it (mirrors trn_agent_boot's slim hook)."""
    import types
    import ctypes
    import contextlib

    try:
        from antenv.axon_hooks import get_axon_ntff_profile_hook  # noqa: F401
        return
    except ImportError:
        pass

    lib = ctypes.CDLL(so_path)
    if not hasattr(lib, "axon_start_nrt_profile"):
        return
    lib.axon_start_nrt_profile.argtypes = [
        ctypes.POINTER(ctypes.c_int64), ctypes.c_size_t,
    ]
    lib.axon_start_nrt_profile.restype = ctypes.c_int64
    lib.axon_stop_nrt_profile.argtypes = [ctypes.c_char_p]
    lib.axon_stop_nrt_profile.restype = ctypes.c_int64

    @contextlib.contextmanager
    def _hook(output_dir, device_ids):
        import jax
        jax.devices()
        if device_ids:
            ids = (ctypes.c_int64 * len(device_ids))(*device_ids)
            rc = lib.axon_start_nrt_profile(ids, len(device_ids))
        else:
            rc = lib.axon_start_nrt_profile(None, 0)
        if rc != 0:
            raise RuntimeError(f"axon_start_nrt_profile rc={rc}")
        try:
            yield
        finally:
            n = lib.axon_stop_nrt_profile(str(output_dir).encode())
            print(f"ntff profile: {n} file(s) written to {output_dir}",
                  file=sys.stderr)

    import antenv
    mod = types.ModuleType("antenv.axon_hooks")
    mod.get_axon_ntff_profile_hook = lambda: _hook
    mod.set_axon_ntff_profile_hook = lambda h: None
    sys.modules["antenv.axon_hooks"] = mod
    antenv.axon_hooks = mod


def kernel(l_fmap, r_fmap, max_disp):
    global LAST_RESULTS
    assert int(max_disp) == MAXD
    lr = _host_prep(l_fmap, r_fmap)

    nc = _get_nc()
    in_maps = [
        {"lr": np.ascontiguousarray(lr[k])} for k in range(NCORES)
    ]

    import os
    trace = bool(int(os.environ.get("CV_TRACE", "0")))
    if trace:
        _install_ntff_hook_shim()
    res = run_bass_kernel_spmd(nc, in_maps, list(range(NCORES)), trace=trace)
    LAST_RESULTS = res

    out = np.empty((B, D, H, W), dtype=np.float32)
    for k in range(NCORES):
        o = np.ascontiguousarray(np.asarray(res.results[k]["o"]))
        s = o.strides  # [B, NHG, NM, 32, 4, SBW] f16
        # v7[b, hg, m, i, t, dk, q] = o[b, hg, m, i, t, 4*i + 4*dk + q]
        v7 = np.lib.stride_tricks.as_strided(
            o,
            shape=(B, NHG, NM, 32, 4, D, 4),
            strides=(s[0], s[1], s[2], s[3] + 4 * s[5], s[4],
                     4 * s[5], s[5]),
        )
        # out[b, 96-dk, 32k + 4hg + t, 128q + 32m + i] = v7[b,hg,m,i,t,dk,q]
        tmp = v7.transpose(0, 5, 1, 4, 6, 2, 3)[:, ::-1]
        out[:, :, HS * k:HS * (k + 1), :] = tmp.reshape(B, D, HS, W)
    return out


# revision 9
# speedup vs baseline: 1.1541x; 1.1497x over previous
"""CostVolume2D Trainium2 kernel (v2).

out[b, d, h, w] = mean_c l[b,c,h,w] * r[b,c,h, w - (d - maxd)]   (zero padded)

Strategy (8 NeuronCores, shard H — no halo since shifts only touch W):
  * Per (b, h): the 97 disparity planes are the diagonals of banded gram
    blocks G_q[i, n] = sum_c l[c, 128q+i] r_pad[c, 128q-48+n], n in [0,224).
  * Tensor engine: 4 matmuls per h (K=64 channels, M=128 w's, N=224).
  * Eviction (PSUM -> SBUF, f32 -> f16) INTERLEAVES the 4 q-blocks:
    g[i, 896*t + 4n + q] = G_q[i, n] for h-in-group t. Row i's 388 valid
    values (the 97-band of all 4 q's) then occupy CONSECUTIVE columns
    [4i, 4i+388) of its 896-column block. Evictions alternate DVE /
    Activation engines to halve per-engine time.
  * Store: per 32-row sub-block m, the valid windows of rows
    [32m, 32m+32) all sit inside columns [128m, 128m+512) — a plain
    rectangular slice. One 3-dim DMA per (group, m) writes
    [32 rows x 4 h x 512] with 1024B descriptors: 16.8 MB/core instead
    of the 29.4 MB full-gram store (valid data is 12.7 MB).
  * Input: l and zero-padded r are packed per row as [l 512 | r_pad 608]
    and laid out so each partition (hh, c) reads its 16 rows contiguously:
    one load DMA per b with 35,840-byte descriptors.
  * Host pre-divides l by C (exact, power of two); host unshard is a
    strided view + transpose (pure layout glue).
"""

import sys

try:
    import concourse  # noqa: F401
except ImportError:
    sys.path.insert(0, "/opt/trn_rl_repo")

import numpy as np

from concourse import bass, mybir
from concourse import tile
from concourse.ap import AP
from concourse.bass_utils import run_bass_kernel_spmd

F32 = mybir.dt.float32
F16 = mybir.dt.float16

# Problem dims (hardcoded per spec)
B, C, H, W = 4, 64, 256, 512
MAXD = 48
D = 2 * MAXD + 1          # 97 disparity planes
NCORES = 8
HS = H // NCORES          # 32 h-rows per core

# Tiling
NH4 = HS // 2             # 16 h-pairs per core (partition dim packs hh in 2)
WROW = W + W + 2 * MAXD   # 1120: [l 512 | r_pad 608] per (c, h) row
NHG = HS // 4             # 8 groups of 4 h per g-tile
GPW = 4 * 224             # 896 interleaved gram columns per h
NM = 4                    # 32-row store sub-blocks
SBW = 512                 # stored row width per sub-block (388 valid + skew)

LAST_RESULTS = None
_NC_CACHE = {}


def _build_nc():
    nc = bass.Bass()
    lr_in = nc.dram_tensor("lr", [B, 2, C, NH4, WROW], F16, kind="ExternalInput")
    o_out = nc.dram_tensor(
        "o", [B, NHG, NM, 32, 8, 256], F16, kind="ExternalOutput"
    )
    lrw = NH4 * WROW      # 17920 free width of lr tile

    with tile.TileContext(nc) as tc:
        with (
            tc.tile_pool(name="lrpool", bufs=2) as lrp,
            tc.tile_pool(name="gpool", bufs=4) as gp,
            tc.tile_pool(name="ppool", bufs=8, space="PSUM") as pp,
        ):
            for b in range(B):
                lr_t = lrp.tile([128, lrw], F16, name="lr_t")
                nc.scalar.dma_start(
                    out=lr_t[:, :],
                    in_=AP(lr_in, b * 2 * C * lrw, [(lrw, 128), (1, lrw)]),
                )
                for hg in range(NHG):
                    g = gp.tile([128, 4 * GPW], F16, name="g", tag="g")
                    for t in range(4):
                        h4 = 2 * hg + (t >> 1)
                        hh = t & 1
                        cb = h4 * WROW
                        for qp in range(2):
                            # single-bank PSUM tile per q-pair; the 2 matmuls
                            # write it interleaved (col = 2n + qq) so the
                            # eviction is one contiguous f32->f16 copy.
                            p_t = pp.tile([128, 448], F32, name="p_t")
                            for qq in range(2):
                                q = 2 * qp + qq
                                lhsT = lr_t[
                                    64 * hh:64 * hh + 64,
                                    cb + 128 * q:cb + 128 * q + 128,
                                ]
                                rhs = lr_t[
                                    64 * hh:64 * hh + 64,
                                    cb + W + 128 * q:cb + W + 128 * q + 224,
                                ]
                                nc.tensor.matmul(
                                    AP(p_t.tensor, qq, [(448, 128), (2, 224)]),
                                    lhsT, rhs, start=True, stop=True,
                                )
                            tb = 2 * t + qp
                            if qp == 0:
                                nc.vector.tensor_copy(
                                    g[:, 448 * tb:448 * (tb + 1)], p_t[:, :]
                                )
                            else:
                                nc.scalar.copy(
                                    g[:, 448 * tb:448 * (tb + 1)], p_t[:, :]
                                )
                    for m in range(NM):
                        nc.sync.dma_start(
                            out=AP(
                                o_out,
                                ((b * NHG + hg) * NM + m) * 32 * 8 * 256,
                                [(8 * 256, 32), (256, 8), (1, 256)],
                            ),
                            in_=AP(
                                g.tensor,
                                m * (32 * 4 * GPW + 64),
                                [(4 * GPW, 32), (448, 8), (1, 256)],
                            ),
                        )
    _split_multi_waits(nc)
    return nc


def _split_multi_waits(nc):
    """The 64-byte TPB instruction encoding holds a single semaphore wait;
    walrus codegen rejects instructions whose sync_info carries more. Hoist
    all but one wait onto standalone InstEventSemaphore instructions placed
    immediately before, on the same engine (FIFO order preserves semantics).
    """
    for bb in nc.main_func.blocks:
        new_list = []
        changed = False
        for ins in bb.instructions:
            si = ins.sync_info
            if si is not None and len(si.on_wait) > 1:
                for w in list(si.on_wait)[:-1]:
                    ev = mybir.InstEventSemaphore(
                        name=nc.get_next_instruction_name(),
                        engine=ins.engine,
                        ins=[],
                        outs=[],
                        sync_info=mybir.SyncInfo(on_wait=[w], on_update=[]),
                    )
                    new_list.append(ev)
                ins.sync_info = mybir.SyncInfo(
                    on_wait=[list(si.on_wait)[-1]], on_update=list(si.on_update)
                )
                changed = True
            new_list.append(ins)
        if changed:
            bb.instructions = new_list


def _get_nc():
    if "nc" not in _NC_CACHE:
        _NC_CACHE["nc"] = _build_nc()
    return _NC_CACHE["nc"]


def _host_prep(l_fmap, r_fmap):
    l = np.asarray(l_fmap, dtype=np.float32) * np.float32(1.0 / C)
    r = np.asarray(r_fmap, dtype=np.float32)
    # per-core layout [k, b, hh, c, h4, col]; h_global = 32k + 2*h4 + hh
    lr = np.zeros((NCORES, B, 2, C, NH4, WROW), dtype=np.float16)
    l6 = l.reshape(B, C, NCORES, NH4, 2, W).transpose(2, 0, 4, 1, 3, 5)
    r6 = r.reshape(B, C, NCORES, NH4, 2, W).transpose(2, 0, 4, 1, 3, 5)
    lr[..., 0:W] = l6
    lr[..., W + MAXD:W + MAXD + W] = r6
    return lr


def _install_ntff_hook_shim(so_path="/opt/axon/libaxon_pjrt.so"):
    """Provide antenv.axon_hooks.get_axon_ntff_profile_hook via ctypes when
    the image's antenv lacks it (mirrors trn_agent_boot's slim hook)."""
    import types
    import ctypes
    import contextlib

    try:
        from antenv.axon_hooks import get_axon_ntff_profile_hook  # noqa: F401
        return
    except ImportError:
        pass

    lib = ctypes.CDLL(so_path)
    if not hasattr(lib, "axon_start_nrt_profile"):
        return
    lib.axon_start_nrt_profile.argtypes = [
        ctypes.POINTER(ctypes.c_int64), ctypes.c_size_t,
    ]
    lib.axon_start_nrt_profile.restype = ctypes.c_int64
    lib.axon_stop_nrt_profile.argtypes = [ctypes.c_char_p]
    lib.axon_stop_nrt_profile.restype = ctypes.c_int64

    @contextlib.contextmanager
    def _hook(output_dir, device_ids):
        import jax
        jax.devices()
        if device_ids:
            ids = (ctypes.c_int64 * len(device_ids))(*device_ids)
            rc = lib.axon_start_nrt_profile(ids, len(device_ids))
        else:
            rc = lib.axon_start_nrt_profile(None, 0)
        if rc != 0:
            raise RuntimeError(f"axon_start_nrt_profile rc={rc}")
        try:
            yield
        finally:
            n = lib.axon_stop_nrt_profile(str(output_dir).encode())
            print(f"ntff profile: {n} file(s) written to {output_dir}",
                  file=sys.stderr)

    import antenv
    mod = types.ModuleType("antenv.axon_hooks")
    mod.get_axon_ntff_profile_hook = lambda: _hook
    mod.set_axon_ntff_profile_hook = lambda h: None
    sys.modules["antenv.axon_hooks"] = mod
    antenv.axon_hooks = mod


def kernel(l_fmap, r_fmap, max_disp):
    global LAST_RESULTS
    assert int(max_disp) == MAXD
    lr = _host_prep(l_fmap, r_fmap)

    nc = _get_nc()
    in_maps = [
        {"lr": np.ascontiguousarray(lr[k])} for k in range(NCORES)
    ]

    import os
    trace = bool(int(os.environ.get("CV_TRACE", "0")))
    if trace:
        _install_ntff_hook_shim()
    res = run_bass_kernel_spmd(nc, in_maps, list(range(NCORES)), trace=trace)
    LAST_RESULTS = res

    out = np.empty((B, D, H, W), dtype=np.float32)
    for k in range(NCORES):
        o = np.ascontiguousarray(np.asarray(res.results[k]["o"]))
        s = o.strides  # [B, NHG, NM, 32, 8, 256] f16
        # v9[b, hg, m, i, t, qp, dk, qq] = o[b, hg, m, i, 2t+qp, 2i+2dk+qq]
        v9 = np.lib.stride_tricks.as_strided(
            o,
            shape=(B, NHG, NM, 32, 4, 2, D, 2),
            strides=(s[0], s[1], s[2], s[3] + 2 * s[5], 2 * s[4], s[4],
                     2 * s[5], s[5]),
        )
        # out[b, 96-dk, 32k + 4hg + t, 256qp + 128qq + 32m + i] = v9[...]
        tmp = v9.transpose(0, 6, 1, 4, 5, 7, 2, 3)[:, ::-1]
        out[:, :, HS * k:HS * (k + 1), :] = tmp.reshape(B, D, HS, W)
    return out


# revision 12
# speedup vs baseline: 1.3927x; 1.2067x over previous
"""CostVolume2D Trainium2 kernel (v2).

out[b, d, h, w] = mean_c l[b,c,h,w] * r[b,c,h, w - (d - maxd)]   (zero padded)

Strategy (8 NeuronCores, shard H — no halo since shifts only touch W):
  * Per (b, h): the 97 disparity planes are the diagonals of banded gram
    blocks G_q[i, n] = sum_c l[c, 128q+i] r_pad[c, 128q-48+n], n in [0,224).
  * Tensor engine: 4 matmuls per h (K=64 channels, M=128 w's, N=224).
  * Eviction (PSUM -> SBUF, f32 -> f16) INTERLEAVES the 4 q-blocks:
    g[i, 896*t + 4n + q] = G_q[i, n] for h-in-group t. Row i's 388 valid
    values (the 97-band of all 4 q's) then occupy CONSECUTIVE columns
    [4i, 4i+388) of its 896-column block. Evictions alternate DVE /
    Activation engines to halve per-engine time.
  * Store: per 32-row sub-block m, the valid windows of rows
    [32m, 32m+32) all sit inside columns [128m, 128m+512) — a plain
    rectangular slice. One 3-dim DMA per (group, m) writes
    [32 rows x 4 h x 512] with 1024B descriptors: 16.8 MB/core instead
    of the 29.4 MB full-gram store (valid data is 12.7 MB).
  * Input: l and zero-padded r are packed per row as [l 512 | r_pad 608]
    and laid out so each partition (hh, c) reads its 16 rows contiguously:
    one load DMA per b with 35,840-byte descriptors.
  * Host pre-divides l by C (exact, power of two); host unshard is a
    strided view + transpose (pure layout glue).
"""

import sys

try:
    import concourse  # noqa: F401
except ImportError:
    sys.path.insert(0, "/opt/trn_rl_repo")

import numpy as np

from concourse import bass, mybir
from concourse import tile
from concourse.ap import AP
from concourse.bass_utils import run_bass_kernel_spmd

F32 = mybir.dt.float32
F16 = mybir.dt.float16

# Problem dims (hardcoded per spec)
B, C, H, W = 4, 64, 256, 512
MAXD = 48
D = 2 * MAXD + 1          # 97 disparity planes
NCORES = 8
HS = H // NCORES          # 32 h-rows per core

# Tiling
NH4 = HS // 2             # 16 h-pairs per core (partition dim packs hh in 2)
WROW = W + W + 2 * MAXD   # 1120: [l 512 | r_pad 608] per (c, h) row
NHG = HS // 4             # 8 groups of 4 h per g-tile
GPW = 4 * 224             # 896 interleaved gram columns per h
NM = 4                    # 32-row store sub-blocks
SBW = 512                 # stored row width per sub-block (388 valid + skew)

LAST_RESULTS = None
_NC_CACHE = {}


def _build_nc():
    nc = bass.Bass()
    lr_in = nc.dram_tensor("lr", [B, 2, C, NH4, WROW], F16, kind="ExternalInput")
    o_out = nc.dram_tensor(
        "o", [B, NHG, NM, 32, 8, 256], F16, kind="ExternalOutput"
    )
    lrw = NH4 * WROW      # 17920 free width of lr tile

    with tile.TileContext(nc) as tc:
        with (
            tc.tile_pool(name="lrpool", bufs=3) as lrp,
            tc.tile_pool(name="gpool", bufs=4) as gp,
            tc.tile_pool(name="ppool", bufs=8, space="PSUM") as pp,
        ):
            lr_tiles = {}

            def emit_load(b):
                # halved loads (h-groups 0-3 / 4-7) so compute starts after
                # the first half lands; issued on Pool (SWDGE) which runs
                # ahead of the busy compute/store engines -> prefetch.
                lr_t = lrp.tile([128, lrw], F16, name="lr_t")
                lr_tiles[b] = lr_t
                half = lrw // 2
                for hf in range(2):
                    nc.gpsimd.dma_start(
                        out=lr_t[:, half * hf:half * (hf + 1)],
                        in_=AP(
                            lr_in, b * 2 * C * lrw + half * hf,
                            [(lrw, 128), (1, half)],
                        ),
                    )

            emit_load(0)
            for b in range(B):
                if b + 1 < B:
                    emit_load(b + 1)
                lr_t = lr_tiles[b]
                for hg in range(NHG):
                    g = gp.tile([128, 4 * GPW], F16, name="g", tag="g")
                    for t in range(4):
                        h4 = 2 * hg + (t >> 1)
                        hh = t & 1
                        cb = h4 * WROW
                        for qp in range(2):
                            # single-bank PSUM tile per q-pair; the 2 matmuls
                            # write it interleaved (col = 2n + qq) so the
                            # eviction is one contiguous f32->f16 copy.
                            p_t = pp.tile([128, 448], F32, name="p_t")
                            for qq in range(2):
                                q = 2 * qp + qq
                                lhsT = lr_t[
                                    64 * hh:64 * hh + 64,
                                    cb + 128 * q:cb + 128 * q + 128,
                                ]
                                rhs = lr_t[
                                    64 * hh:64 * hh + 64,
                                    cb + W + 128 * q:cb + W + 128 * q + 224,
                                ]
                                nc.tensor.matmul(
                                    AP(p_t.tensor, qq, [(448, 128), (2, 224)]),
                                    lhsT, rhs, start=True, stop=True,
                                )
                            tb = 2 * t + qp
                            if qp == 0:
                                nc.vector.tensor_copy(
                                    g[:, 448 * tb:448 * (tb + 1)], p_t[:, :]
                                )
                            else:
                                nc.scalar.copy(
                                    g[:, 448 * tb:448 * (tb + 1)], p_t[:, :]
                                )
                    for m in range(NM):
                        # split store issue between SP and Pool (SWDGE):
                        # each dma_start costs the issuing sequencer ~0.6-1us
                        eng = nc.sync if m % 2 == 0 else nc.gpsimd
                        eng.dma_start(
                            out=AP(
                                o_out,
                                ((b * NHG + hg) * NM + m) * 32 * 8 * 256,
                                [(8 * 256, 32), (256, 8), (1, 256)],
                            ),
                            in_=AP(
                                g.tensor,
                                m * (32 * 4 * GPW + 64),
                                [(4 * GPW, 32), (448, 8), (1, 256)],
                            ),
                        )
    _split_multi_waits(nc)
    return nc


def _split_multi_waits(nc):
    """The 64-byte TPB instruction encoding holds a single semaphore wait;
    walrus codegen rejects instructions whose sync_info carries more. Hoist
    all but one wait onto standalone InstEventSemaphore instructions placed
    immediately before, on the same engine (FIFO order preserves semantics).
    """
    for bb in nc.main_func.blocks:
        new_list = []
        changed = False
        for ins in bb.instructions:
            si = ins.sync_info
            if si is not None and len(si.on_wait) > 1:
                for w in list(si.on_wait)[:-1]:
                    ev = mybir.InstEventSemaphore(
                        name=nc.get_next_instruction_name(),
                        engine=ins.engine,
                        ins=[],
                        outs=[],
                        sync_info=mybir.SyncInfo(on_wait=[w], on_update=[]),
                    )
                    new_list.append(ev)
                ins.sync_info = mybir.SyncInfo(
                    on_wait=[list(si.on_wait)[-1]], on_update=list(si.on_update)
                )
                changed = True
            new_list.append(ins)
        if changed:
            bb.instructions = new_list


def _get_nc():
    if "nc" not in _NC_CACHE:
        _NC_CACHE["nc"] = _build_nc()
    return _NC_CACHE["nc"]


def _host_prep(l_fmap, r_fmap):
    l = np.asarray(l_fmap, dtype=np.float32) * np.float32(1.0 / C)
    r = np.asarray(r_fmap, dtype=np.float32)
    # per-core layout [k, b, hh, c, h4, col]; h_global = 32k + 2*h4 + hh
    lr = np.zeros((NCORES, B, 2, C, NH4, WROW), dtype=np.float16)
    l6 = l.reshape(B, C, NCORES, NH4, 2, W).transpose(2, 0, 4, 1, 3, 5)
    r6 = r.reshape(B, C, NCORES, NH4, 2, W).transpose(2, 0, 4, 1, 3, 5)
    lr[..., 0:W] = l6
    lr[..., W + MAXD:W + MAXD + W] = r6
    return lr


def _install_ntff_hook_shim(so_path="/opt/axon/libaxon_pjrt.so"):
    """Provide antenv.axon_hooks.get_axon_ntff_profile_hook via ctypes when
    the image's antenv lacks it (mirrors trn_agent_boot's slim hook)."""
    import types
    import ctypes
    import contextlib

    try:
        from antenv.axon_hooks import get_axon_ntff_profile_hook  # noqa: F401
        return
    except ImportError:
        pass

    lib = ctypes.CDLL(so_path)
    if not hasattr(lib, "axon_start_nrt_profile"):
        return
    lib.axon_start_nrt_profile.argtypes = [
        ctypes.POINTER(ctypes.c_int64), ctypes.c_size_t,
    ]
    lib.axon_start_nrt_profile.restype = ctypes.c_int64
    lib.axon_stop_nrt_profile.argtypes = [ctypes.c_char_p]
    lib.axon_stop_nrt_profile.restype = ctypes.c_int64

    @contextlib.contextmanager
    def _hook(output_dir, device_ids):
        import jax
        jax.devices()
        if device_ids:
            ids = (ctypes.c_int64 * len(device_ids))(*device_ids)
            rc = lib.axon_start_nrt_profile(ids, len(device_ids))
        else:
            rc = lib.axon_start_nrt_profile(None, 0)
        if rc != 0:
            raise RuntimeError(f"axon_start_nrt_profile rc={rc}")
        try:
            yield
        finally:
            n = lib.axon_stop_nrt_profile(str(output_dir).encode())
            print(f"ntff profile: {n} file(s) written to {output_dir}",
                  file=sys.stderr)

    import antenv
    mod = types.ModuleType("antenv.axon_hooks")
    mod.get_axon_ntff_profile_hook = lambda: _hook
    mod.set_axon_ntff_profile_hook = lambda h: None
    sys.modules["antenv.axon_hooks"] = mod
    antenv.axon_hooks = mod


def kernel(l_fmap, r_fmap, max_disp):
    global LAST_RESULTS
    assert int(max_disp) == MAXD
    lr = _host_prep(l_fmap, r_fmap)

    nc = _get_nc()
    in_maps = [
        {"lr": np.ascontiguousarray(lr[k])} for k in range(NCORES)
    ]

    import os
    trace = bool(int(os.environ.get("CV_TRACE", "0")))
    if trace:
        _install_ntff_hook_shim()
    res = run_bass_kernel_spmd(nc, in_maps, list(range(NCORES)), trace=trace)
    LAST_RESULTS = res

    out = np.empty((B, D, H, W), dtype=np.float32)
    for k in range(NCORES):
        o = np.ascontiguousarray(np.asarray(res.results[k]["o"]))
        s = o.strides  # [B, NHG, NM, 32, 8, 256] f16
        # v9[b, hg, m, i, t, qp, dk, qq] = o[b, hg, m, i, 2t+qp, 2i+2dk+qq]
        v9 = np.lib.stride_tricks.as_strided(
            o,
            shape=(B, NHG, NM, 32, 4, 2, D, 2),
            strides=(s[0], s[1], s[2], s[3] + 2 * s[5], 2 * s[4], s[4],
                     2 * s[5], s[5]),
        )
        # out[b, 96-dk, 32k + 4hg + t, 256qp + 128qq + 32m + i] = v9[...]
        tmp = v9.transpose(0, 6, 1, 4, 5, 7, 2, 3)[:, ::-1]
        out[:, :, HS * k:HS * (k + 1), :] = tmp.reshape(B, D, HS, W)
    return out


# revision 14
# speedup vs baseline: 1.4458x; 1.0381x over previous
"""CostVolume2D Trainium2 kernel (v2).

out[b, d, h, w] = mean_c l[b,c,h,w] * r[b,c,h, w - (d - maxd)]   (zero padded)

Strategy (8 NeuronCores, shard H — no halo since shifts only touch W):
  * Per (b, h): the 97 disparity planes are the diagonals of banded gram
    blocks G_q[i, n] = sum_c l[c, 128q+i] r_pad[c, 128q-48+n], n in [0,224).
  * Tensor engine: 4 matmuls per h (K=64 channels, M=128 w's, N=224).
  * Eviction (PSUM -> SBUF, f32 -> f16) INTERLEAVES the 4 q-blocks:
    g[i, 896*t + 4n + q] = G_q[i, n] for h-in-group t. Row i's 388 valid
    values (the 97-band of all 4 q's) then occupy CONSECUTIVE columns
    [4i, 4i+388) of its 896-column block. Evictions alternate DVE /
    Activation engines to halve per-engine time.
  * Store: per 32-row sub-block m, the valid windows of rows
    [32m, 32m+32) all sit inside columns [128m, 128m+512) — a plain
    rectangular slice. One 3-dim DMA per (group, m) writes
    [32 rows x 4 h x 512] with 1024B descriptors: 16.8 MB/core instead
    of the 29.4 MB full-gram store (valid data is 12.7 MB).
  * Input: l and zero-padded r are packed per row as [l 512 | r_pad 608]
    and laid out so each partition (hh, c) reads its 16 rows contiguously:
    one load DMA per b with 35,840-byte descriptors.
  * Host pre-divides l by C (exact, power of two); host unshard is a
    strided view + transpose (pure layout glue).
"""

import sys

try:
    import concourse  # noqa: F401
except ImportError:
    sys.path.insert(0, "/opt/trn_rl_repo")

import numpy as np

from concourse import bass, mybir
from concourse import tile
from concourse.ap import AP
from concourse.bass_utils import run_bass_kernel_spmd

F32 = mybir.dt.float32
F16 = mybir.dt.float16

# Problem dims (hardcoded per spec)
B, C, H, W = 4, 64, 256, 512
MAXD = 48
D = 2 * MAXD + 1          # 97 disparity planes
NCORES = 8
HS = H // NCORES          # 32 h-rows per core

# Tiling
NH4 = HS // 2             # 16 h-pairs per core (partition dim packs hh in 2)
WROW = W + W + 2 * MAXD   # 1120: [l 512 | r_pad 608] per (c, h) row
NHG = HS // 4             # 8 groups of 4 h per g-tile
GPW = 4 * 224             # 896 interleaved gram columns per h
NM = 4                    # 32-row store sub-blocks
SBW = 512                 # stored row width per sub-block (388 valid + skew)

LAST_RESULTS = None
_NC_CACHE = {}


def _build_nc():
    nc = bass.Bass()
    lr_in = nc.dram_tensor("lr", [B, 2, C, NH4, WROW], F16, kind="ExternalInput")
    o_out = nc.dram_tensor(
        "o", [B, NHG, NM, 32, 8, 256], F16, kind="ExternalOutput"
    )
    lrw = NH4 * WROW      # 17920 free width of lr tile

    with tile.TileContext(nc) as tc:
        with (
            tc.tile_pool(name="lrpool", bufs=3) as lrp,
            tc.tile_pool(name="gpool", bufs=8) as gp,
            tc.tile_pool(name="ppool", bufs=8, space="PSUM") as pp,
        ):
            lr_tiles = {}
            qw = lrw // 4

            def emit_load_quarter(b, qt):
                # quarter loads (2 h-groups each), issued on Pool (SWDGE)
                # which runs ahead of the busy compute/store engines and
                # spread across the previous b's compute -> smooth prefetch.
                if b >= B:
                    return
                if b not in lr_tiles:
                    lr_tiles[b] = lrp.tile([128, lrw], F16, name="lr_t")
                lr_t = lr_tiles[b]
                nc.gpsimd.dma_start(
                    out=lr_t[:, qw * qt:qw * (qt + 1)],
                    in_=AP(
                        lr_in, b * 2 * C * lrw + qw * qt,
                        [(lrw, 128), (1, qw)],
                    ),
                )

            for qt in range(4):
                emit_load_quarter(0, qt)
            for b in range(B):
                lr_t = lr_tiles[b]
                for hg in range(NHG):
                    if hg % 2 == 0:
                        emit_load_quarter(b + 1, hg // 2)
                    g = gp.tile([128, 4 * GPW], F16, name="g", tag="g")
                    for t in range(4):
                        h4 = 2 * hg + (t >> 1)
                        hh = t & 1
                        cb = h4 * WROW
                        for qp in range(2):
                            # single-bank PSUM tile per q-pair; the 2 matmuls
                            # write it interleaved (col = 2n + qq) so the
                            # eviction is one contiguous f32->f16 copy.
                            p_t = pp.tile([128, 448], F32, name="p_t")
                            for qq in range(2):
                                q = 2 * qp + qq
                                lhsT = lr_t[
                                    64 * hh:64 * hh + 64,
                                    cb + 128 * q:cb + 128 * q + 128,
                                ]
                                rhs = lr_t[
                                    64 * hh:64 * hh + 64,
                                    cb + W + 128 * q:cb + W + 128 * q + 224,
                                ]
                                nc.tensor.matmul(
                                    AP(p_t.tensor, qq, [(448, 128), (2, 224)]),
                                    lhsT, rhs, start=True, stop=True,
                                )
                            tb = 2 * t + qp
                            if qp == 0:
                                nc.vector.tensor_copy(
                                    g[:, 448 * tb:448 * (tb + 1)], p_t[:, :]
                                )
                            else:
                                nc.scalar.copy(
                                    g[:, 448 * tb:448 * (tb + 1)], p_t[:, :]
                                )
                    for m in range(NM):
                        # split store issue between SP and Pool (SWDGE):
                        # each dma_start costs the issuing sequencer ~0.6-1us
                        eng = nc.sync if m % 2 == 0 else nc.gpsimd
                        eng.dma_start(
                            out=AP(
                                o_out,
                                ((b * NHG + hg) * NM + m) * 32 * 8 * 256,
                                [(8 * 256, 32), (256, 8), (1, 256)],
                            ),
                            in_=AP(
                                g.tensor,
                                m * (32 * 4 * GPW + 64),
                                [(4 * GPW, 32), (448, 8), (1, 256)],
                            ),
                        )
    _split_multi_waits(nc)
    return nc


def _split_multi_waits(nc):
    """The 64-byte TPB instruction encoding holds a single semaphore wait;
    walrus codegen rejects instructions whose sync_info carries more. Hoist
    all but one wait onto standalone InstEventSemaphore instructions placed
    immediately before, on the same engine (FIFO order preserves semantics).
    """
    for bb in nc.main_func.blocks:
        new_list = []
        changed = False
        for ins in bb.instructions:
            si = ins.sync_info
            if si is not None and len(si.on_wait) > 1:
                for w in list(si.on_wait)[:-1]:
                    ev = mybir.InstEventSemaphore(
                        name=nc.get_next_instruction_name(),
                        engine=ins.engine,
                        ins=[],
                        outs=[],
                        sync_info=mybir.SyncInfo(on_wait=[w], on_update=[]),
                    )
                    new_list.append(ev)
                ins.sync_info = mybir.SyncInfo(
                    on_wait=[list(si.on_wait)[-1]], on_update=list(si.on_update)
                )
                changed = True
            new_list.append(ins)
        if changed:
            bb.instructions = new_list


def _get_nc():
    if "nc" not in _NC_CACHE:
        _NC_CACHE["nc"] = _build_nc()
    return _NC_CACHE["nc"]


def _host_prep(l_fmap, r_fmap):
    l = np.asarray(l_fmap, dtype=np.float32) * np.float32(1.0 / C)
    r = np.asarray(r_fmap, dtype=np.float32)
    # per-core layout [k, b, hh, c, h4, col]; h_global = 32k + 2*h4 + hh
    lr = np.zeros((NCORES, B, 2, C, NH4, WROW), dtype=np.float16)
    l6 = l.reshape(B, C, NCORES, NH4, 2, W).transpose(2, 0, 4, 1, 3, 5)
    r6 = r.reshape(B, C, NCORES, NH4, 2, W).transpose(2, 0, 4, 1, 3, 5)
    lr[..., 0:W] = l6
    lr[..., W + MAXD:W + MAXD + W] = r6
    return lr


def _install_ntff_hook_shim(so_path="/opt/axon/libaxon_pjrt.so"):
    """Provide antenv.axon_hooks.get_axon_ntff_profile_hook via ctypes when
    the image's antenv lacks it (mirrors trn_agent_boot's slim hook)."""
    import types
    import ctypes
    import contextlib

    try:
        from antenv.axon_hooks import get_axon_ntff_profile_hook  # noqa: F401
        return
    except ImportError:
        pass

    lib = ctypes.CDLL(so_path)
    if not hasattr(lib, "axon_start_nrt_profile"):
        return
    lib.axon_start_nrt_profile.argtypes = [
        ctypes.POINTER(ctypes.c_int64), ctypes.c_size_t,
    ]
    lib.axon_start_nrt_profile.restype = ctypes.c_int64
    lib.axon_stop_nrt_profile.argtypes = [ctypes.c_char_p]
    lib.axon_stop_nrt_profile.restype = ctypes.c_int64

    @contextlib.contextmanager
    def _hook(output_dir, device_ids):
        import jax
        jax.devices()
        if device_ids:
            ids = (ctypes.c_int64 * len(device_ids))(*device_ids)
            rc = lib.axon_start_nrt_profile(ids, len(device_ids))
        else:
            rc = lib.axon_start_nrt_profile(None, 0)
        if rc != 0:
            raise RuntimeError(f"axon_start_nrt_profile rc={rc}")
        try:
            yield
        finally:
            n = lib.axon_stop_nrt_profile(str(output_dir).encode())
            print(f"ntff profile: {n} file(s) written to {output_dir}",
                  file=sys.stderr)

    import antenv
    mod = types.ModuleType("antenv.axon_hooks")
    mod.get_axon_ntff_profile_hook = lambda: _hook
    mod.set_axon_ntff_profile_hook = lambda h: None
    sys.modules["antenv.axon_hooks"] = mod
    antenv.axon_hooks = mod


def kernel(l_fmap, r_fmap, max_disp):
    global LAST_RESULTS
    assert int(max_disp) == MAXD
    lr = _host_prep(l_fmap, r_fmap)

    nc = _get_nc()
    in_maps = [
        {"lr": np.ascontiguousarray(lr[k])} for k in range(NCORES)
    ]

    import os
    trace = bool(int(os.environ.get("CV_TRACE", "0")))
    if trace:
        _install_ntff_hook_shim()
    res = run_bass_kernel_spmd(nc, in_maps, list(range(NCORES)), trace=trace)
    LAST_RESULTS = res

    out = np.empty((B, D, H, W), dtype=np.float32)
    for k in range(NCORES):
        o = np.ascontiguousarray(np.asarray(res.results[k]["o"]))
        s = o.strides  # [B, NHG, NM, 32, 8, 256] f16
        # v9[b, hg, m, i, t, qp, dk, qq] = o[b, hg, m, i, 2t+qp, 2i+2dk+qq]
        v9 = np.lib.stride_tricks.as_strided(
            o,
            shape=(B, NHG, NM, 32, 4, 2, D, 2),
            strides=(s[0], s[1], s[2], s[3] + 2 * s[5], 2 * s[4], s[4],
                     2 * s[5], s[5]),
        )
        # out[b, 96-dk, 32k + 4hg + t, 256qp + 128qq + 32m + i] = v9[...]
        tmp = v9.transpose(0, 6, 1, 4, 5, 7, 2, 3)[:, ::-1]
        out[:, :, HS * k:HS * (k + 1), :] = tmp.reshape(B, D, HS, W)
    return out


# revision 16
# speedup vs baseline: 1.4497x; 1.0027x over previous
"""CostVolume2D Trainium2 kernel (v2).

out[b, d, h, w] = mean_c l[b,c,h,w] * r[b,c,h, w - (d - maxd)]   (zero padded)

Strategy (8 NeuronCores, shard H — no halo since shifts only touch W):
  * Per (b, h): the 97 disparity planes are the diagonals of banded gram
    blocks G_q[i, n] = sum_c l[c, 128q+i] r_pad[c, 128q-48+n], n in [0,224).
  * Tensor engine: 4 matmuls per h (K=64 channels, M=128 w's, N=224).
  * Eviction (PSUM -> SBUF, f32 -> f16) INTERLEAVES the 4 q-blocks:
    g[i, 896*t + 4n + q] = G_q[i, n] for h-in-group t. Row i's 388 valid
    values (the 97-band of all 4 q's) then occupy CONSECUTIVE columns
    [4i, 4i+388) of its 896-column block. Evictions alternate DVE /
    Activation engines to halve per-engine time.
  * Store: per 32-row sub-block m, the valid windows of rows
    [32m, 32m+32) all sit inside columns [128m, 128m+512) — a plain
    rectangular slice. One 3-dim DMA per (group, m) writes
    [32 rows x 4 h x 512] with 1024B descriptors: 16.8 MB/core instead
    of the 29.4 MB full-gram store (valid data is 12.7 MB).
  * Input: l and zero-padded r are packed per row as [l 512 | r_pad 608]
    and laid out so each partition (hh, c) reads its 16 rows contiguously:
    one load DMA per b with 35,840-byte descriptors.
  * Host pre-divides l by C (exact, power of two); host unshard is a
    strided view + transpose (pure layout glue).
"""

import sys

try:
    import concourse  # noqa: F401
except ImportError:
    sys.path.insert(0, "/opt/trn_rl_repo")

import numpy as np

from concourse import bass, mybir
from concourse import tile
from concourse.ap import AP
from concourse.bass_utils import run_bass_kernel_spmd

F32 = mybir.dt.float32
F16 = mybir.dt.float16

# Problem dims (hardcoded per spec)
B, C, H, W = 4, 64, 256, 512
MAXD = 48
D = 2 * MAXD + 1          # 97 disparity planes
NCORES = 8
HS = H // NCORES          # 32 h-rows per core

# Tiling
NH4 = HS // 2             # 16 h-pairs per core (partition dim packs hh in 2)
WROW = W + W + 2 * MAXD   # 1120: [l 512 | r_pad 608] per (c, h) row
NHG = HS // 4             # 8 groups of 4 h per g-tile
GPW = 4 * 224             # 896 interleaved gram columns per h
NM = 4                    # 32-row store sub-blocks
SBW = 512                 # stored row width per sub-block (388 valid + skew)

LAST_RESULTS = None
_NC_CACHE = {}


def _build_nc():
    nc = bass.Bass()
    lr_in = nc.dram_tensor("lr", [B, 2, C, NH4, WROW], F16, kind="ExternalInput")
    o_out = nc.dram_tensor(
        "o", [B, NHG, NM, 32, 8, 256], F16, kind="ExternalOutput"
    )
    lrw = NH4 * WROW      # 17920 free width of lr tile

    with tile.TileContext(nc) as tc:
        with (
            tc.tile_pool(name="lrpool", bufs=3) as lrp,
            tc.tile_pool(name="gpool", bufs=8) as gp,
            tc.tile_pool(name="ppool", bufs=8, space="PSUM") as pp,
        ):
            lr_tiles = {}
            qw = lrw // 4

            def emit_load_quarter(b, qt):
                # quarter loads (2 h-groups each), issued on Pool (SWDGE)
                # which runs ahead of the busy compute/store engines and
                # spread across the previous b's compute -> smooth prefetch.
                if b >= B:
                    return
                if b not in lr_tiles:
                    lr_tiles[b] = lrp.tile([128, lrw], F16, name="lr_t")
                lr_t = lr_tiles[b]
                nc.gpsimd.dma_start(
                    out=lr_t[:, qw * qt:qw * (qt + 1)],
                    in_=AP(
                        lr_in, b * 2 * C * lrw + qw * qt,
                        [(lrw, 128), (1, qw)],
                    ),
                )

            for qt in range(4):
                emit_load_quarter(0, qt)
            for b in range(B):
                lr_t = lr_tiles[b]
                for hg in range(NHG):
                    if hg % 2 == 1:
                        emit_load_quarter(b + 1, hg // 2)
                    g = gp.tile([128, 4 * GPW], F16, name="g", tag="g")
                    for t in range(4):
                        h4 = 2 * hg + (t >> 1)
                        hh = t & 1
                        cb = h4 * WROW
                        for qp in range(2):
                            # single-bank PSUM tile per q-pair; the 2 matmuls
                            # write it interleaved (col = 2n + qq) so the
                            # eviction is one contiguous f32->f16 copy.
                            p_t = pp.tile([128, 448], F32, name="p_t")
                            for qq in range(2):
                                q = 2 * qp + qq
                                lhsT = lr_t[
                                    64 * hh:64 * hh + 64,
                                    cb + 128 * q:cb + 128 * q + 128,
                                ]
                                rhs = lr_t[
                                    64 * hh:64 * hh + 64,
                                    cb + W + 128 * q:cb + W + 128 * q + 224,
                                ]
                                nc.tensor.matmul(
                                    AP(p_t.tensor, qq, [(448, 128), (2, 224)]),
                                    lhsT, rhs, start=True, stop=True,
                                )
                            tb = 2 * t + qp
                            if qp == 0:
                                nc.vector.tensor_copy(
                                    g[:, 448 * tb:448 * (tb + 1)], p_t[:, :]
                                )
                            else:
                                nc.scalar.copy(
                                    g[:, 448 * tb:448 * (tb + 1)], p_t[:, :]
                                )
                    for m in range(NM):
                        # split store issue between SP and Pool (SWDGE):
                        # each dma_start costs the issuing sequencer ~0.6-1us
                        eng = (
                            nc.sync if (hg * NM + m) % 8 < 3 else nc.gpsimd
                        )
                        eng.dma_start(
                            out=AP(
                                o_out,
                                ((b * NHG + hg) * NM + m) * 32 * 8 * 256,
                                [(8 * 256, 32), (256, 8), (1, 256)],
                            ),
                            in_=AP(
                                g.tensor,
                                m * (32 * 4 * GPW + 64),
                                [(4 * GPW, 32), (448, 8), (1, 256)],
                            ),
                        )
    _split_multi_waits(nc)
    return nc


def _split_multi_waits(nc):
    """The 64-byte TPB instruction encoding holds a single semaphore wait;
    walrus codegen rejects instructions whose sync_info carries more. Hoist
    all but one wait onto standalone InstEventSemaphore instructions placed
    immediately before, on the same engine (FIFO order preserves semantics).
    """
    for bb in nc.main_func.blocks:
        new_list = []
        changed = False
        for ins in bb.instructions:
            si = ins.sync_info
            if si is not None and len(si.on_wait) > 1:
                for w in list(si.on_wait)[:-1]:
                    ev = mybir.InstEventSemaphore(
                        name=nc.get_next_instruction_name(),
                        engine=ins.engine,
                        ins=[],
                        outs=[],
                        sync_info=mybir.SyncInfo(on_wait=[w], on_update=[]),
                    )
                    new_list.append(ev)
                ins.sync_info = mybir.SyncInfo(
                    on_wait=[list(si.on_wait)[-1]], on_update=list(si.on_update)
                )
                changed = True
            new_list.append(ins)
        if changed:
            bb.instructions = new_list


def _get_nc():
    if "nc" not in _NC_CACHE:
        _NC_CACHE["nc"] = _build_nc()
    return _NC_CACHE["nc"]


def _host_prep(l_fmap, r_fmap):
    l = np.asarray(l_fmap, dtype=np.float32) * np.float32(1.0 / C)
    r = np.asarray(r_fmap, dtype=np.float32)
    # per-core layout [k, b, hh, c, h4, col]; h_global = 32k + 2*h4 + hh
    lr = np.zeros((NCORES, B, 2, C, NH4, WROW), dtype=np.float16)
    l6 = l.reshape(B, C, NCORES, NH4, 2, W).transpose(2, 0, 4, 1, 3, 5)
    r6 = r.reshape(B, C, NCORES, NH4, 2, W).transpose(2, 0, 4, 1, 3, 5)
    lr[..., 0:W] = l6
    lr[..., W + MAXD:W + MAXD + W] = r6
    return lr


def _install_ntff_hook_shim(so_path="/opt/axon/libaxon_pjrt.so"):
    """Provide antenv.axon_hooks.get_axon_ntff_profile_hook via ctypes when
    the image's antenv lacks it (mirrors trn_agent_boot's slim hook)."""
    import types
    import ctypes
    import contextlib

    try:
        from antenv.axon_hooks import get_axon_ntff_profile_hook  # noqa: F401
        return
    except ImportError:
        pass

    lib = ctypes.CDLL(so_path)
    if not hasattr(lib, "axon_start_nrt_profile"):
        return
    lib.axon_start_nrt_profile.argtypes = [
        ctypes.POINTER(ctypes.c_int64), ctypes.c_size_t,
    ]
    lib.axon_start_nrt_profile.restype = ctypes.c_int64
    lib.axon_stop_nrt_profile.argtypes = [ctypes.c_char_p]
    lib.axon_stop_nrt_profile.restype = ctypes.c_int64

    @contextlib.contextmanager
    def _hook(output_dir, device_ids):
        import jax
        jax.devices()
        if device_ids:
            ids = (ctypes.c_int64 * len(device_ids))(*device_ids)
            rc = lib.axon_start_nrt_profile(ids, len(device_ids))
        else:
            rc = lib.axon_start_nrt_profile(None, 0)
        if rc != 0:
            raise RuntimeError(f"axon_start_nrt_profile rc={rc}")
        try:
            yield
        finally:
            n = lib.axon_stop_nrt_profile(str(output_dir).encode())
            print(f"ntff profile: {n} file(s) written to {output_dir}",
                  file=sys.stderr)

    import antenv
    mod = types.ModuleType("antenv.axon_hooks")
    mod.get_axon_ntff_profile_hook = lambda: _hook
    mod.set_axon_ntff_profile_hook = lambda h: None
    sys.modules["antenv.axon_hooks"] = mod
    antenv.axon_hooks = mod


def kernel(l_fmap, r_fmap, max_disp):
    global LAST_RESULTS
    assert int(max_disp) == MAXD
    lr = _host_prep(l_fmap, r_fmap)

    nc = _get_nc()
    in_maps = [
        {"lr": np.ascontiguousarray(lr[k])} for k in range(NCORES)
    ]

    import os
    trace = bool(int(os.environ.get("CV_TRACE", "0")))
    if trace:
        _install_ntff_hook_shim()
    res = run_bass_kernel_spmd(nc, in_maps, list(range(NCORES)), trace=trace)
    LAST_RESULTS = res

    out = np.empty((B, D, H, W), dtype=np.float32)
    for k in range(NCORES):
        o = np.ascontiguousarray(np.asarray(res.results[k]["o"]))
        s = o.strides  # [B, NHG, NM, 32, 8, 256] f16
        # v9[b, hg, m, i, t, qp, dk, qq] = o[b, hg, m, i, 2t+qp, 2i+2dk+qq]
        v9 = np.lib.stride_tricks.as_strided(
            o,
            shape=(B, NHG, NM, 32, 4, 2, D, 2),
            strides=(s[0], s[1], s[2], s[3] + 2 * s[5], 2 * s[4], s[4],
                     2 * s[5], s[5]),
        )
        # out[b, 96-dk, 32k + 4hg + t, 256qp + 128qq + 32m + i] = v9[...]
        tmp = v9.transpose(0, 6, 1, 4, 5, 7, 2, 3)[:, ::-1]
        out[:, :, HS * k:HS * (k + 1), :] = tmp.reshape(B, D, HS, W)
    return out


# revision 22
# speedup vs baseline: 1.4784x; 1.0197x over previous
"""CostVolume2D Trainium2 kernel (v2).

out[b, d, h, w] = mean_c l[b,c,h,w] * r[b,c,h, w - (d - maxd)]   (zero padded)

Strategy (8 NeuronCores, shard H — no halo since shifts only touch W):
  * Per (b, h): the 97 disparity planes are the diagonals of banded gram
    blocks G_q[i, n] = sum_c l[c, 128q+i] r_pad[c, 128q-48+n], n in [0,224).
  * Tensor engine: 4 matmuls per h (K=64 channels, M=128 w's, N=224).
  * Eviction (PSUM -> SBUF, f32 -> f16) INTERLEAVES the 4 q-blocks:
    g[i, 896*t + 4n + q] = G_q[i, n] for h-in-group t. Row i's 388 valid
    values (the 97-band of all 4 q's) then occupy CONSECUTIVE columns
    [4i, 4i+388) of its 896-column block. Evictions alternate DVE /
    Activation engines to halve per-engine time.
  * Store: per 32-row sub-block m, the valid windows of rows
    [32m, 32m+32) all sit inside columns [128m, 128m+512) — a plain
    rectangular slice. One 3-dim DMA per (group, m) writes
    [32 rows x 4 h x 512] with 1024B descriptors: 16.8 MB/core instead
    of the 29.4 MB full-gram store (valid data is 12.7 MB).
  * Input: l and zero-padded r are packed per row as [l 512 | r_pad 608]
    and laid out so each partition (hh, c) reads its 16 rows contiguously:
    one load DMA per b with 35,840-byte descriptors.
  * Host pre-divides l by C (exact, power of two); host unshard is a
    strided view + transpose (pure layout glue).
"""

import sys

try:
    import concourse  # noqa: F401
except ImportError:
    sys.path.insert(0, "/opt/trn_rl_repo")

import numpy as np

from concourse import bass, mybir
from concourse import tile
from concourse.ap import AP
from concourse.bass_utils import run_bass_kernel_spmd

F32 = mybir.dt.float32
F16 = mybir.dt.float16

# Problem dims (hardcoded per spec)
B, C, H, W = 4, 64, 256, 512
MAXD = 48
D = 2 * MAXD + 1          # 97 disparity planes
NCORES = 8
HS = H // NCORES          # 32 h-rows per core

# Tiling
NH4 = HS // 2             # 16 h-pairs per core (partition dim packs hh in 2)
WROW = 2 * W              # 1024: [l 512 | r 512] per (c, h) row (no zero pad:
                          # edge matmuls are shortened and the host zeroes
                          # the out-of-image disparities afterwards)
NHG = HS // 4             # 8 groups of 4 h per g-tile
GPW = 4 * 224             # 896 interleaved gram columns per h
NM = 4                    # 32-row store sub-blocks
SBW = 512                 # stored row width per sub-block (388 valid + skew)

LAST_RESULTS = None
_NC_CACHE = {}


def _build_nc():
    nc = bass.Bass()
    lr_in = nc.dram_tensor("lr", [B, 2, C, NH4, WROW], F16, kind="ExternalInput")
    o_out = nc.dram_tensor(
        "o", [B, NHG, NM, 32, 8, 256], F16, kind="ExternalOutput"
    )
    lrw = NH4 * WROW      # 17920 free width of lr tile

    with tile.TileContext(nc) as tc:
        with (
            tc.tile_pool(name="lrpool", bufs=3) as lrp,
            tc.tile_pool(name="gpool", bufs=8) as gp,
            tc.tile_pool(name="ppool", bufs=8, space="PSUM") as pp,
        ):
            lr_tiles = {}
            qw = lrw // 4

            def emit_load(b, frac, nfrac):
                # partial loads (nfrac-th of a b), issued on Pool (SWDGE)
                # which runs ahead of the busy compute/store engines and
                # spread across the previous b's compute -> smooth prefetch.
                if b >= B:
                    return
                if b not in lr_tiles:
                    lr_tiles[b] = lrp.tile([128, lrw], F16, name="lr_t")
                lr_t = lr_tiles[b]
                fw = lrw // nfrac
                nc.gpsimd.dma_start(
                    out=lr_t[:, fw * frac:fw * (frac + 1)],
                    in_=AP(
                        lr_in, b * 2 * C * lrw + fw * frac,
                        [(lrw, 128), (1, fw)],
                    ),
                )

            for et in range(8):
                emit_load(0, et, 8)
            for b in range(B):
                lr_t = lr_tiles[b]
                for hg in range(NHG):
                    if hg % 2 == 1:
                        emit_load(b + 1, hg // 2, 4)
                    g = gp.tile([128, 4 * GPW], F16, name="g", tag="g")
                    for t in range(4):
                        h4 = 2 * hg + (t >> 1)
                        hh = t & 1
                        cb = h4 * WROW
                        for qp in range(2):
                            # single-bank PSUM tile per q-pair; the 2 matmuls
                            # write it interleaved (col = 2n + qq) so the
                            # eviction is one contiguous f32->f16 copy.
                            # Edge blocks (q=0/q=3) use shortened rhs windows
                            # (the zero pad is dropped from the input); the
                            # uncovered psum slots hold stale data that maps
                            # to out-of-image disparities, zeroed on host.
                            p_t = pp.tile([128, 448], F32, name="p_t")
                            for qq in range(2):
                                q = 2 * qp + qq
                                lhsT = lr_t[
                                    64 * hh:64 * hh + 64,
                                    cb + 128 * q:cb + 128 * q + 128,
                                ]
                                r0 = max(0, 128 * q - MAXD)
                                r1 = min(W, 128 * q + 128 + MAXD)
                                rhs = lr_t[
                                    64 * hh:64 * hh + 64,
                                    cb + W + r0:cb + W + r1,
                                ]
                                # psum col j = 2n + qq, n = r-col - (128q-48)
                                joff = 2 * (r0 - (128 * q - MAXD))
                                nc.tensor.matmul(
                                    AP(
                                        p_t.tensor, qq + joff,
                                        [(448, 128), (2, r1 - r0)],
                                    ),
                                    lhsT, rhs, start=True, stop=True,
                                )
                            tb = 2 * t + qp
                            if qp == 0:
                                nc.vector.tensor_copy(
                                    g[:, 448 * tb:448 * (tb + 1)], p_t[:, :]
                                )
                            else:
                                nc.scalar.copy(
                                    g[:, 448 * tb:448 * (tb + 1)], p_t[:, :]
                                )
                    for m in range(NM):
                        # split store issue between SP and Pool (SWDGE):
                        # each dma_start costs the issuing sequencer ~0.6-1us
                        eng = (
                            nc.sync if (hg * NM + m) % 8 < 3 else nc.gpsimd
                        )
                        eng.dma_start(
                            out=AP(
                                o_out,
                                ((b * NHG + hg) * NM + m) * 32 * 8 * 256,
                                [(8 * 256, 32), (256, 8), (1, 256)],
                            ),
                            in_=AP(
                                g.tensor,
                                m * (32 * 4 * GPW + 64),
                                [(4 * GPW, 32), (448, 8), (1, 256)],
                            ),
                        )
    _split_multi_waits(nc)
    return nc


def _split_multi_waits(nc):
    """The 64-byte TPB instruction encoding holds a single semaphore wait;
    walrus codegen rejects instructions whose sync_info carries more. Hoist
    all but one wait onto standalone InstEventSemaphore instructions placed
    immediately before, on the same engine (FIFO order preserves semantics).
    """
    for bb in nc.main_func.blocks:
        new_list = []
        changed = False
        for ins in bb.instructions:
            si = ins.sync_info
            if si is not None and len(si.on_wait) > 1:
                for w in list(si.on_wait)[:-1]:
                    ev = mybir.InstEventSemaphore(
                        name=nc.get_next_instruction_name(),
                        engine=ins.engine,
                        ins=[],
                        outs=[],
                        sync_info=mybir.SyncInfo(on_wait=[w], on_update=[]),
                    )
                    new_list.append(ev)
                ins.sync_info = mybir.SyncInfo(
                    on_wait=[list(si.on_wait)[-1]], on_update=list(si.on_update)
                )
                changed = True
            new_list.append(ins)
        if changed:
            bb.instructions = new_list


def _get_nc():
    if "nc" not in _NC_CACHE:
        _NC_CACHE["nc"] = _build_nc()
    return _NC_CACHE["nc"]


def _host_prep(l_fmap, r_fmap):
    l = np.asarray(l_fmap, dtype=np.float32) * np.float32(1.0 / C)
    r = np.asarray(r_fmap, dtype=np.float32)
    # per-core layout [k, b, hh, c, h4, col]; h_global = 32k + 2*h4 + hh
    lr = np.empty((NCORES, B, 2, C, NH4, WROW), dtype=np.float16)
    l6 = l.reshape(B, C, NCORES, NH4, 2, W).transpose(2, 0, 4, 1, 3, 5)
    r6 = r.reshape(B, C, NCORES, NH4, 2, W).transpose(2, 0, 4, 1, 3, 5)
    lr[..., 0:W] = l6
    lr[..., W:2 * W] = r6
    return lr


def _install_ntff_hook_shim(so_path="/opt/axon/libaxon_pjrt.so"):
    """Provide antenv.axon_hooks.get_axon_ntff_profile_hook via ctypes when
    the image's antenv lacks it (mirrors trn_agent_boot's slim hook)."""
    import types
    import ctypes
    import contextlib

    try:
        from antenv.axon_hooks import get_axon_ntff_profile_hook  # noqa: F401
        return
    except ImportError:
        pass

    lib = ctypes.CDLL(so_path)
    if not hasattr(lib, "axon_start_nrt_profile"):
        return
    lib.axon_start_nrt_profile.argtypes = [
        ctypes.POINTER(ctypes.c_int64), ctypes.c_size_t,
    ]
    lib.axon_start_nrt_profile.restype = ctypes.c_int64
    lib.axon_stop_nrt_profile.argtypes = [ctypes.c_char_p]
    lib.axon_stop_nrt_profile.restype = ctypes.c_int64

    @contextlib.contextmanager
    def _hook(output_dir, device_ids):
        import jax
        jax.devices()
        if device_ids:
            ids = (ctypes.c_int64 * len(device_ids))(*device_ids)
            rc = lib.axon_start_nrt_profile(ids, len(device_ids))
        else:
            rc = lib.axon_start_nrt_profile(None, 0)
        if rc != 0:
            raise RuntimeError(f"axon_start_nrt_profile rc={rc}")
        try:
            yield
        finally:
            n = lib.axon_stop_nrt_profile(str(output_dir).encode())
            print(f"ntff profile: {n} file(s) written to {output_dir}",
                  file=sys.stderr)

    import antenv
    mod = types.ModuleType("antenv.axon_hooks")
    mod.get_axon_ntff_profile_hook = lambda: _hook
    mod.set_axon_ntff_profile_hook = lambda h: None
    sys.modules["antenv.axon_hooks"] = mod
    antenv.axon_hooks = mod


def kernel(l_fmap, r_fmap, max_disp):
    global LAST_RESULTS
    assert int(max_disp) == MAXD
    lr = _host_prep(l_fmap, r_fmap)

    nc = _get_nc()
    in_maps = [
        {"lr": np.ascontiguousarray(lr[k])} for k in range(NCORES)
    ]

    import os
    trace = bool(int(os.environ.get("CV_TRACE", "0")))
    if trace:
        _install_ntff_hook_shim()
    res = run_bass_kernel_spmd(nc, in_maps, list(range(NCORES)), trace=trace)
    LAST_RESULTS = res

    out = np.empty((B, D, H, W), dtype=np.float32)
    for k in range(NCORES):
        o = np.ascontiguousarray(np.asarray(res.results[k]["o"]))
        s = o.strides  # [B, NHG, NM, 32, 8, 256] f16
        # v9[b, hg, m, i, t, qp, dk, qq] = o[b, hg, m, i, 2t+qp, 2i+2dk+qq]
        v9 = np.lib.stride_tricks.as_strided(
            o,
            shape=(B, NHG, NM, 32, 4, 2, D, 2),
            strides=(s[0], s[1], s[2], s[3] + 2 * s[5], 2 * s[4], s[4],
                     2 * s[5], s[5]),
        )
        # out[b, 96-dk, 32k + 4hg + t, 256qp + 128qq + 32m + i] = v9[...]
        tmp = v9.transpose(0, 6, 1, 4, 5, 7, 2, 3)[:, ::-1]
        out[:, :, HS * k:HS * (k + 1), :] = tmp.reshape(B, D, HS, W)
    # out-of-image disparities (reference zero padding); on-device these
    # slots hold stale PSUM data since the edge matmuls are shortened
    for w in range(MAXD):
        out[:, w + MAXD + 1:, :, w] = 0.0
    for w in range(W - MAXD, W):
        out[:, :w - (W - MAXD - 1), :, w] = 0.0
    return out


# revision 23
# speedup vs baseline: 1.6470x; 1.1141x over previous
"""CostVolume2D Trainium2 kernel (v2).

out[b, d, h, w] = mean_c l[b,c,h,w] * r[b,c,h, w - (d - maxd)]   (zero padded)

Strategy (8 NeuronCores, shard H — no halo since shifts only touch W):
  * Per (b, h): the 97 disparity planes are the diagonals of banded gram
    blocks G_q[i, n] = sum_c l[c, 128q+i] r_pad[c, 128q-48+n], n in [0,224).
  * Tensor engine: 4 matmuls per h (K=64 channels, M=128 w's, N=224).
  * Eviction (PSUM -> SBUF, f32 -> f16) INTERLEAVES the 4 q-blocks:
    g[i, 896*t + 4n + q] = G_q[i, n] for h-in-group t. Row i's 388 valid
    values (the 97-band of all 4 q's) then occupy CONSECUTIVE columns
    [4i, 4i+388) of its 896-column block. Evictions alternate DVE /
    Activation engines to halve per-engine time.
  * Store: per 32-row sub-block m, the valid windows of rows
    [32m, 32m+32) all sit inside columns [128m, 128m+512) — a plain
    rectangular slice. One 3-dim DMA per (group, m) writes
    [32 rows x 4 h x 512] with 1024B descriptors: 16.8 MB/core instead
    of the 29.4 MB full-gram store (valid data is 12.7 MB).
  * Input: l and zero-padded r are packed per row as [l 512 | r_pad 608]
    and laid out so each partition (hh, c) reads its 16 rows contiguously:
    one load DMA per b with 35,840-byte descriptors.
  * Host pre-divides l by C (exact, power of two); host unshard is a
    strided view + transpose (pure layout glue).
"""

import sys

try:
    import concourse  # noqa: F401
except ImportError:
    sys.path.insert(0, "/opt/trn_rl_repo")

import numpy as np

from concourse import bass, mybir
from concourse import tile
from concourse.ap import AP
from concourse.bass_utils import run_bass_kernel_spmd

F32 = mybir.dt.float32
F16 = mybir.dt.float16

# Problem dims (hardcoded per spec)
B, C, H, W = 4, 64, 256, 512
MAXD = 48
D = 2 * MAXD + 1          # 97 disparity planes
NCORES = 8
HS = H // NCORES          # 32 h-rows per core

# Tiling
NH4 = HS // 2             # 16 h-pairs per core (partition dim packs hh in 2)
WROW = 2 * W              # 1024: [l 512 | r 512] per (c, h) row (no zero pad:
                          # edge matmuls are shortened and the host zeroes
                          # the out-of-image disparities afterwards)
NHG = HS // 4             # 8 groups of 4 h per g-tile
GPW = 4 * 224             # 896 interleaved gram columns per h
NM = 4                    # 32-row store sub-blocks
SBW = 512                 # stored row width per sub-block (388 valid + skew)

LAST_RESULTS = None
_NC_CACHE = {}


def _build_nc():
    nc = bass.Bass()
    lr_in = nc.dram_tensor("lr", [B, 2, C, NH4, WROW], F16, kind="ExternalInput")
    o_out = nc.dram_tensor(
        "o", [B, NHG, NM, 32, 8, 256], F16, kind="ExternalOutput"
    )
    lrw = NH4 * WROW      # 17920 free width of lr tile

    with tile.TileContext(nc) as tc:
        with (
            tc.tile_pool(name="lrpool", bufs=3) as lrp,
            tc.tile_pool(name="gpool", bufs=8) as gp,
            tc.tile_pool(name="ppool", bufs=8, space="PSUM") as pp,
        ):
            lr_tiles = {}
            qw = lrw // 4

            def emit_load(b, frac, nfrac):
                # partial loads (nfrac-th of a b), issued on Pool (SWDGE)
                # which runs ahead of the busy compute/store engines and
                # spread across the previous b's compute -> smooth prefetch.
                if b >= B:
                    return
                if b not in lr_tiles:
                    lr_tiles[b] = lrp.tile([128, lrw], F16, name="lr_t")
                lr_t = lr_tiles[b]
                fw = lrw // nfrac
                nc.gpsimd.dma_start(
                    out=lr_t[:, fw * frac:fw * (frac + 1)],
                    in_=AP(
                        lr_in, b * 2 * C * lrw + fw * frac,
                        [(lrw, 128), (1, fw)],
                    ),
                )

            for et in range(8):
                emit_load(0, et, 8)
            for b in range(B):
                lr_t = lr_tiles[b]
                for hg in range(NHG):
                    emit_load(b + 1, hg, 8)
                    g = gp.tile([128, 4 * GPW], F16, name="g", tag="g")
                    for t in range(4):
                        h4 = 2 * hg + (t >> 1)
                        hh = t & 1
                        cb = h4 * WROW
                        for qp in range(2):
                            # single-bank PSUM tile per q-pair; the 2 matmuls
                            # write it interleaved (col = 2n + qq) so the
                            # eviction is one contiguous f32->f16 copy.
                            # Edge blocks (q=0/q=3) use shortened rhs windows
                            # (the zero pad is dropped from the input); the
                            # uncovered psum slots hold stale data that maps
                            # to out-of-image disparities, zeroed on host.
                            p_t = pp.tile([128, 448], F32, name="p_t")
                            for qq in range(2):
                                q = 2 * qp + qq
                                lhsT = lr_t[
                                    64 * hh:64 * hh + 64,
                                    cb + 128 * q:cb + 128 * q + 128,
                                ]
                                r0 = max(0, 128 * q - MAXD)
                                r1 = min(W, 128 * q + 128 + MAXD)
                                rhs = lr_t[
                                    64 * hh:64 * hh + 64,
                                    cb + W + r0:cb + W + r1,
                                ]
                                # psum col j = 2n + qq, n = r-col - (128q-48)
                                joff = 2 * (r0 - (128 * q - MAXD))
                                nc.tensor.matmul(
                                    AP(
                                        p_t.tensor, qq + joff,
                                        [(448, 128), (2, r1 - r0)],
                                    ),
                                    lhsT, rhs, start=True, stop=True,
                                )
                            tb = 2 * t + qp
                            if qp == 0:
                                nc.vector.tensor_copy(
                                    g[:, 448 * tb:448 * (tb + 1)], p_t[:, :]
                                )
                            else:
                                nc.scalar.copy(
                                    g[:, 448 * tb:448 * (tb + 1)], p_t[:, :]
                                )
                    for m in range(NM):
                        # split store issue between SP and Pool (SWDGE):
                        # each dma_start costs the issuing sequencer ~0.6-1us
                        eng = (
                            nc.sync if (hg * NM + m) % 8 < 3 else nc.gpsimd
                        )
                        eng.dma_start(
                            out=AP(
                                o_out,
                                ((b * NHG + hg) * NM + m) * 32 * 8 * 256,
                                [(8 * 256, 32), (256, 8), (1, 256)],
                            ),
                            in_=AP(
                                g.tensor,
                                m * (32 * 4 * GPW + 64),
                                [(4 * GPW, 32), (448, 8), (1, 256)],
                            ),
                        )
    _split_multi_waits(nc)
    return nc


def _split_multi_waits(nc):
    """The 64-byte TPB instruction encoding holds a single semaphore wait;
    walrus codegen rejects instructions whose sync_info carries more. Hoist
    all but one wait onto standalone InstEventSemaphore instructions placed
    immediately before, on the same engine (FIFO order preserves semantics).
    """
    for bb in nc.main_func.blocks:
        new_list = []
        changed = False
        for ins in bb.instructions:
            si = ins.sync_info
            if si is not None and len(si.on_wait) > 1:
                for w in list(si.on_wait)[:-1]:
                    ev = mybir.InstEventSemaphore(
                        name=nc.get_next_instruction_name(),
                        engine=ins.engine,
                        ins=[],
                        outs=[],
                        sync_info=mybir.SyncInfo(on_wait=[w], on_update=[]),
                    )
                    new_list.append(ev)
                ins.sync_info = mybir.SyncInfo(
                    on_wait=[list(si.on_wait)[-1]], on_update=list(si.on_update)
                )
                changed = True
            new_list.append(ins)
        if changed:
            bb.instructions = new_list


def _get_nc():
    if "nc" not in _NC_CACHE:
        _NC_CACHE["nc"] = _build_nc()
    return _NC_CACHE["nc"]


def _host_prep(l_fmap, r_fmap):
    l = np.asarray(l_fmap, dtype=np.float32) * np.float32(1.0 / C)
    r = np.asarray(r_fmap, dtype=np.float32)
    # per-core layout [k, b, hh, c, h4, col]; h_global = 32k + 2*h4 + hh
    lr = np.empty((NCORES, B, 2, C, NH4, WROW), dtype=np.float16)
    l6 = l.reshape(B, C, NCORES, NH4, 2, W).transpose(2, 0, 4, 1, 3, 5)
    r6 = r.reshape(B, C, NCORES, NH4, 2, W).transpose(2, 0, 4, 1, 3, 5)
    lr[..., 0:W] = l6
    lr[..., W:2 * W] = r6
    return lr


def _install_ntff_hook_shim(so_path="/opt/axon/libaxon_pjrt.so"):
    """Provide antenv.axon_hooks.get_axon_ntff_profile_hook via ctypes when
    the image's antenv lacks it (mirrors trn_agent_boot's slim hook)."""
    import types
    import ctypes
    import contextlib

    try:
        from antenv.axon_hooks import get_axon_ntff_profile_hook  # noqa: F401
        return
    except ImportError:
        pass

    lib = ctypes.CDLL(so_path)
    if not hasattr(lib, "axon_start_nrt_profile"):
        return
    lib.axon_start_nrt_profile.argtypes = [
        ctypes.POINTER(ctypes.c_int64), ctypes.c_size_t,
    ]
    lib.axon_start_nrt_profile.restype = ctypes.c_int64
    lib.axon_stop_nrt_profile.argtypes = [ctypes.c_char_p]
    lib.axon_stop_nrt_profile.restype = ctypes.c_int64

    @contextlib.contextmanager
    def _hook(output_dir, device_ids):
        import jax
        jax.devices()
        if device_ids:
            ids = (ctypes.c_int64 * len(device_ids))(*device_ids)
            rc = lib.axon_start_nrt_profile(ids, len(device_ids))
        else:
            rc = lib.axon_start_nrt_profile(None, 0)
        if rc != 0:
            raise RuntimeError(f"axon_start_nrt_profile rc={rc}")
        try:
            yield
        finally:
            n = lib.axon_stop_nrt_profile(str(output_dir).encode())
            print(f"ntff profile: {n} file(s) written to {output_dir}",
                  file=sys.stderr)

    import antenv
    mod = types.ModuleType("antenv.axon_hooks")
    mod.get_axon_ntff_profile_hook = lambda: _hook
    mod.set_axon_ntff_profile_hook = lambda h: None
    sys.modules["antenv.axon_hooks"] = mod
    antenv.axon_hooks = mod


def kernel(l_fmap, r_fmap, max_disp):
    global LAST_RESULTS
    assert int(max_disp) == MAXD
    lr = _host_prep(l_fmap, r_fmap)

    nc = _get_nc()
    in_maps = [
        {"lr": np.ascontiguousarray(lr[k])} for k in range(NCORES)
    ]

    import os
    trace = bool(int(os.environ.get("CV_TRACE", "0")))
    if trace:
        _install_ntff_hook_shim()
    res = run_bass_kernel_spmd(nc, in_maps, list(range(NCORES)), trace=trace)
    LAST_RESULTS = res

    out = np.empty((B, D, H, W), dtype=np.float32)
    for k in range(NCORES):
        o = np.ascontiguousarray(np.asarray(res.results[k]["o"]))
        s = o.strides  # [B, NHG, NM, 32, 8, 256] f16
        # v9[b, hg, m, i, t, qp, dk, qq] = o[b, hg, m, i, 2t+qp, 2i+2dk+qq]
        v9 = np.lib.stride_tricks.as_strided(
            o,
            shape=(B, NHG, NM, 32, 4, 2, D, 2),
            strides=(s[0], s[1], s[2], s[3] + 2 * s[5], 2 * s[4], s[4],
                     2 * s[5], s[5]),
        )
        # out[b, 96-dk, 32k + 4hg + t, 256qp + 128qq + 32m + i] = v9[...]
        tmp = v9.transpose(0, 6, 1, 4, 5, 7, 2, 3)[:, ::-1]
        out[:, :, HS * k:HS * (k + 1), :] = tmp.reshape(B, D, HS, W)
    # out-of-image disparities (reference zero padding); on-device these
    # slots hold stale PSUM data since the edge matmuls are shortened
    for w in range(MAXD):
        out[:, w + MAXD + 1:, :, w] = 0.0
    for w in range(W - MAXD, W):
        out[:, :w - (W - MAXD - 1), :, w] = 0.0
    return out
